# revision 23
# baseline (speedup 1.0000x reference)
"""AdaAttention distributed Bass kernel for 8 TRN2 NeuronCores (v4).

Module (per batch b):
  xn = instancenorm(x[b]); sn = instancenorm(s[b])
  q = Wq@xn + bq; k = Wk@sn + bk; v = Wv@s[b] + bv     (1x1 convs, [C, N])
  per head h (d=64): attn = softmax(q_h^T k_h / sqrt(d)) over keys
  o_h = v_h @ attn^T;  out = Wo@o + bo + x[b]

Sharding: core i -> b = i//4, group-rank r = i%4, heads {2r, 2r+1}.

The PE on this part is activity-throttled to ~1.2GHz when kept dense, so
the design minimizes STREAMED COLUMNS and, v4, PE *stall time*:
  - all 1x1 convs run fp8 DoubleRow (K=256 per matmul); x/s arrive from
    the host as fp8 channel-pair packs [128, 2, N], weights as fp8 packs
    [128, 2, 128].
  - vT is produced DIRECTLY as matmul(lhsT=s_pack_tile, rhs=wv_pack):
    out [keys, couts] = v^T tile.  This deletes v_sb, the PE transposes
    and the identity matrix.  bv is folded into bo on the host
    (bo_eff = bo + Wo@bv): the v-bias commutes through the softmax
    column-normalization and the Wo conv.
  - attn@V: fp8 DoubleRow K=256; scores: bf16 K=64.
  - o is scaled x64 before fp8 (dodges fp8e4m3 denormals), AllGathered
    in fp8 per 512-col chunk DURING attention; ALL Wo convs are emitted
    AFTER the last attention chunk so a late peer chunk can never stall
    the PE mid-attention (v3 lost ~106us to exactly that).
  - exp split ScalarE (Exp LUT -> fp8) / VectorE (Schraudolph bit trick
    -> fp8e4m3 bits via uint8 write); colsum row in vT normalizes both.
  - instance norm folded into conv weights (rstd per channel-pair slice,
    beff via a DoubleRow matmul against x256-scaled fp8 means).  Stats
    tail ops are emitted at top priority so k/q convs start ~15-25us.
Host sends big contiguous tensors; DMAs are issued across sync/scalar/
gpsimd queues to cut issue serialization.
"""

import numpy as np

B, C, T, J, H = 2, 512, 128, 24, 8
N = T * J                  # 3072
D = C // H                 # 64
NCORES = 8
GRPS = [[0, 1, 2, 3], [4, 5, 6, 7]]
HPC = 2                    # heads per core
CPC = HPC * D              # 128 channels per core
EPS = 1e-5
SCALE = 1.0 / float(np.sqrt(D))   # 1/8

NT = N // 512              # 6 n-chunks of 512
MT = N // 128              # 24 m-tiles of 128
MP = MT // 2               # 12 m-pairs
NG = 2                     # channel pair-groups (2 x (128x2) = 512)
OSC = 64.0                 # o pre-fp8 scale (1/OSC applied after Wo conv)
WSC = 16.0                 # weight pre-fp8 scale (dodges fp8 denormals)
MSC = 256.0                # mean pre-fp8 scale for the beff matmul

# Schraudolph fast-exp constants for fp8e4m3 output bits (HW-calibrated)
EXPA = float(8.0 * SCALE * np.log2(np.e))
EXPB = 55.593
ACT_SHARE = 12             # of the 24 (head, m-pair) exp units per nj


def _build():
    import os

    import concourse.bass as bass
    import concourse.tile as tile
    from concourse import bacc, mybir

    F32 = mybir.dt.float32
    BF16 = mybir.dt.bfloat16
    FP8 = mybir.dt.float8e4
    U8 = mybir.dt.uint8
    DR = mybir.MatmulPerfMode.DoubleRow
    DBG = os.environ.get("KERNEL_DEBUG") == "1"

    nc = bacc.Bacc("TRN2", target_bir_lowering=False, debug=False,
                   num_devices=NCORES)

    x_d = [nc.dram_tensor(f"x{g}", [128, 2, N], FP8, kind="ExternalInput").ap()
           for g in range(NG)]
    s_d = [nc.dram_tensor(f"s{g}", [128, 2, N], FP8, kind="ExternalInput").ap()
           for g in range(NG)]
    xres_d = nc.dram_tensor("xres", [CPC, N], BF16, kind="ExternalInput").ap()
    w_d = {}
    for wname in ("wq", "wk", "wv", "wo"):
        for g in range(NG):
            w_d[(wname, g)] = nc.dram_tensor(
                f"{wname}{g}", [128, 2, CPC], FP8, kind="ExternalInput").ap()
    bq_d = nc.dram_tensor("bq", [CPC, 1], F32, kind="ExternalInput").ap()
    bk_d = nc.dram_tensor("bk", [CPC, 1], F32, kind="ExternalInput").ap()
    bo_d = nc.dram_tensor("bo", [CPC, 1], F32, kind="ExternalInput").ap()
    out_d = nc.dram_tensor("out", [CPC, N], F32, kind="ExternalOutput").ap()
    if DBG:
        dbg_q = nc.dram_tensor("dbg_q", [CPC, N], F32, kind="ExternalOutput").ap()
        dbg_k = nc.dram_tensor("dbg_k", [CPC, N], F32, kind="ExternalOutput").ap()
        dbg_e = nc.dram_tensor("dbg_e", [CPC, N], F32, kind="ExternalOutput").ap()

    with tile.TileContext(nc) as tc:
        from contextlib import ExitStack
        with tc.tile_pool(name="persist", bufs=1) as persist, \
             tc.tile_pool(name="dram", bufs=1, space="DRAM") as dram:
            acts_scope = ExitStack()
            acts = acts_scope.enter_context(tc.tile_pool(name="acts", bufs=1))
            xt = [acts.tile([128, 2, N], FP8, tag=f"xt{g}", name=f"xt{g}")
                  for g in range(NG)]
            st = [acts.tile([128, 2, N], FP8, tag=f"st{g}", name=f"st{g}")
                  for g in range(NG)]
            scr = acts.tile([128, N], BF16, tag="scr", name="scr")
            q_sb = persist.tile([128, N], BF16, tag="q_sb", name="q_sb")
            k_sb = persist.tile([128, N], BF16, tag="k_sb", name="k_sb")
            vT = [[persist.tile([128, 2, 80], FP8, tag=f"vT{h}_{m}",
                                name=f"vT{h}_{m}") for m in range(MP)]
                  for h in range(HPC)]
            o_sb = persist.tile([128, N], FP8, tag="o_sb", name="o_sb")
            xres = persist.tile([128, N], BF16, tag="xres", name="xres")
            ws = {}
            for wname in ("wq", "wk", "wv", "wo"):
                for g in range(NG):
                    ws[(wname, g)] = persist.tile(
                        [128, 2, CPC], FP8, tag=f"{wname}{g}",
                        name=f"{wname}{g}")
            of_sb = [[persist.tile([128, 2, 512], FP8, tag=f"of{nj}_{g}",
                                   name=f"of{nj}_{g}") for g in range(NG)]
                     for nj in range(NT)]
            beff_q = persist.tile([128, 1], F32, tag="beff_q", name="beff_q")
            beff_k = persist.tile([128, 1], F32, tag="beff_k", name="beff_k")
            bo_sb = persist.tile([128, 1], F32, tag="bo_sb", name="bo_sb")
            eps_sb = persist.tile([128, 1], F32, tag="eps_sb", name="eps_sb")
            warm = persist.tile([128, 1], F32, tag="warm", name="warm")

            nc.vector.memset(o_sb[:, 0:16], 0.0)
            nc.vector.memset(eps_sb, EPS)
            nc.vector.memset(warm, 0.0)
            for h in range(HPC):
                for m in range(MP):
                    nc.vector.memset(vT[h][m][:, :, D:D + 1], 1.0)
            # preload the exp table while the pipe fills
            nc.scalar.activation(out=warm, in_=warm,
                                 func=mybir.ActivationFunctionType.Exp,
                                 scale=1.0)

            # tiny warm-up AllGather: absorbs first-collective ncfw setup
            # and core launch skew; its input DMA leads the sync queue
            wu_in = dram.tile([128, 16], FP8, tag="wu_in", name="wu_in")
            wu_out = dram.tile([512, 16], FP8, tag="wu_out", name="wu_out")
            nc.sync.dma_start(out=wu_in, in_=o_sb[:, 0:16])
            nc.gpsimd.collective_compute(
                "AllGather", mybir.AluOpType.bypass, replica_groups=GRPS,
                ins=[wu_in[:].opt()], outs=[wu_out[:].opt()])

            # input DMAs: s (+wv) first on sync, x on scalar, the rest of
            # the weights after s, xres (needed late) on gpsimd.
            for g in range(NG):
                nc.sync.dma_start(out=ws[("wv", g)], in_=w_d[("wv", g)])
            for c in range(NT):
                csl = slice(512 * c, 512 * (c + 1))
                for g in range(NG):
                    nc.sync.dma_start(out=st[g][:, :, csl],
                                      in_=s_d[g][:, :, csl])
                if c % 3 == 0:
                    # x in 1536-col chunks: matches the ACT stats halves
                    # and gives the DMA engines 3x longer rows
                    xsl = slice(512 * c, 512 * (c + 3))
                    for g in range(NG):
                        nc.scalar.dma_start(out=xt[g][:, :, xsl],
                                            in_=x_d[g][:, :, xsl])
            for wname in ("wk", "wq", "wo"):
                for g in range(NG):
                    nc.sync.dma_start(out=ws[(wname, g)], in_=w_d[(wname, g)])

            # ---- stage 1+2: stats || vT build || k conv.  Emission order
            # == per-engine priority for the Tile scheduler, arranged so
            # the stats tail (aggr/sqrt/fold) preempts bulk work.
            vt_scope = ExitStack()
            vps = vt_scope.enter_context(
                tc.tile_pool(name="vt_ps", bufs=2, space="PSUM"))
            conv_scope = ExitStack()
            cps = conv_scope.enter_context(
                tc.tile_pool(name="conv_ps", bufs=3, space="PSUM"))
            stats_scope = ExitStack()
            stats_pool = stats_scope.enter_context(
                tc.tile_pool(name="stats", bufs=2))
            sps = stats_scope.enter_context(
                tc.tile_pool(name="stats_ps", bufs=2, space="PSUM"))

            mean = {}
            var_col = {}
            rstds = {}

            # 1. DVE: s stats via bn_stats, chunk-trailing the s DMA
            for g in range(NG):
                for j in range(2):
                    stt = stats_pool.tile([128, NT, 6], F32, tag="bn",
                                          name="bn")
                    for c in range(NT):
                        nc.vector.bn_stats(
                            out=stt[:, c, :],
                            in_=st[g][:, j, 512 * c:512 * (c + 1)])
                    mv = stats_pool.tile([128, 2], F32, tag=f"mv_s{g}{j}",
                                         name=f"mv_s{g}{j}")
                    nc.vector.bn_aggr(out=mv, in_=stt)
                    mean[("s", g, j)] = mv[:, 0:1]
                    var_col[("s", g, j)] = mv[:, 1:2]

            # 2. ACT: s rstd sqrts at TOP priority (ready ~when aggr lands)
            for gg in range(NG):
                for jj in range(2):
                    rstd = stats_pool.tile(
                        [128, 1], F32, tag=f"rstd_s{gg}{jj}",
                        name=f"rstd_s{gg}{jj}")
                    nc.scalar.activation(
                        out=rstd, in_=var_col[("s", gg, jj)],
                        func=mybir.ActivationFunctionType.Sqrt,
                        bias=eps_sb, scale=1.0)
                    rstds[("s", gg, jj)] = rstd

            # 3. ACT: x stats for 3 units (2-pass accum, 1536-col chunks)
            ACT_UNITS = [(0, 0), (0, 1), (1, 0)]
            xsump = {}
            xsqp = {}
            for g, j in ACT_UNITS:
                sump = stats_pool.tile([128, 2], F32, tag=f"xsump{g}{j}",
                                       name=f"xsump{g}{j}")
                sqp = stats_pool.tile([128, 2], F32, tag=f"xsqp{g}{j}",
                                      name=f"xsqp{g}{j}")
                for c in range(2):
                    csl = slice(1536 * c, 1536 * (c + 1))
                    nc.scalar.activation(
                        out=scr[:, csl], in_=xt[g][:, j, csl],
                        func=mybir.ActivationFunctionType.Square,
                        accum_out=sqp[:, c:c + 1])
                    nc.scalar.activation(
                        out=scr[:, csl], in_=xt[g][:, j, csl],
                        func=mybir.ActivationFunctionType.Copy,
                        accum_out=sump[:, c:c + 1])
                xsump[(g, j)] = sump
                xsqp[(g, j)] = sqp

            # 4. PE: direct vT build: out[keys, couts] = s_tile^T W_v^T.
            #    lhsT = s pack tile (stationary), rhs = wv pack.  Only the
            #    first VT_PRE pairs are built here (PSUM is scarce and the
            #    DVE copies must not delay the stats tail); pairs VT_PRE..
            #    are interleaved into attention chunk 0, whose AV consumers
            #    trail by ~3 pairs.
            VT_PRE = 2

            def emit_vt_pair(mp):
                pt = vps.tile([128, 2, 128], F32, tag="vt", name="vt")
                for j in range(2):
                    m = 2 * mp + j
                    msl = slice(128 * m, 128 * (m + 1))
                    for g in range(NG):
                        nc.tensor.matmul(pt[:, j, :],
                                         lhsT=st[g][:, :, msl],
                                         rhs=ws[("wv", g)], perf_mode=DR,
                                         start=(g == 0), stop=(g == NG - 1))
                return pt

            def emit_vt_copy(mp, pt):
                for h in range(HPC):
                    nc.vector.tensor_scalar_mul(
                        vT[h][mp][:, :, 0:D], pt[:, :, D * h:D * (h + 1)],
                        1.0 / WSC)

            vt_pts = {mp: emit_vt_pair(mp) for mp in range(VT_PRE)}

            # 5. DVE: s folds + fp8 means; then k conv + biases
            mean_f8 = {}
            for name in ("s", "x"):
                for g in range(NG):
                    mean_f8[(name, g)] = stats_pool.tile(
                        [128, 2, 1], FP8, tag=f"mf_{name}{g}",
                        name=f"mf_{name}{g}")
            for g in range(NG):
                for j in range(2):
                    rstd = rstds[("s", g, j)]
                    nc.vector.reciprocal(out=rstd, in_=rstd)
                    w = ws[("wk", g)]
                    nc.vector.tensor_scalar_mul(w[:, j, :], w[:, j, :], rstd)
                    nc.vector.tensor_scalar_mul(
                        mean_f8[("s", g)][:, j, :], mean[("s", g, j)], MSC)

            # 6. PE: beff_k matmul, then k conv
            mps_k = sps.tile([128, 1], F32, tag="mps", name="mps")
            for g in range(NG):
                nc.tensor.matmul(mps_k, lhsT=ws[("wk", g)],
                                 rhs=mean_f8[("s", g)], perf_mode=DR,
                                 start=(g == 0), stop=(g == NG - 1))
            nc.sync.dma_start(out=beff_k, in_=bk_d[:, :])
            nc.vector.scalar_tensor_tensor(
                out=beff_k, in0=mps_k, scalar=-1.0 / (WSC * MSC), in1=beff_k,
                op0=mybir.AluOpType.mult, op1=mybir.AluOpType.add)
            for nj in range(NT):
                nsl = slice(512 * nj, 512 * (nj + 1))
                pk = cps.tile([128, 512], F32, tag="conv", name="conv")
                for g in range(NG):
                    nc.tensor.matmul(pk, lhsT=ws[("wk", g)],
                                     rhs=st[g][:, :, nsl], perf_mode=DR,
                                     start=(g == 0), stop=(g == NG - 1))
                nc.vector.tensor_scalar(
                    out=k_sb[:, nsl], in0=pk, scalar1=1.0 / WSC,
                    scalar2=beff_k, op0=mybir.AluOpType.mult,
                    op1=mybir.AluOpType.add)

            # 7. x combines (DVE) + x11 stats (DVE) + Sqrt_x (ACT) + folds
            stt = stats_pool.tile([128, NT, 6], F32, tag="bnx", name="bnx")
            for c in range(NT):
                nc.vector.bn_stats(out=stt[:, c, :],
                                   in_=xt[1][:, 1, 512 * c:512 * (c + 1)])
            mv_x11 = stats_pool.tile([128, 2], F32, tag="mv_x11",
                                     name="mv_x11")
            nc.vector.bn_aggr(out=mv_x11, in_=stt)
            mean[("x", 1, 1)] = mv_x11[:, 0:1]
            var_col[("x", 1, 1)] = mv_x11[:, 1:2]

            for g in range(NG):
                for j in range(2):
                    if (g, j) in ACT_UNITS:
                        mv = stats_pool.tile([128, 2], F32,
                                             tag=f"mv_x{g}{j}",
                                             name=f"mv_x{g}{j}")
                        sump, sqp = xsump[(g, j)], xsqp[(g, j)]
                        nc.vector.tensor_add(mv[:, 0:1], sump[:, 0:1],
                                             sump[:, 1:2])
                        nc.vector.tensor_scalar_mul(mv[:, 0:1], mv[:, 0:1],
                                                    1.0 / N)
                        nc.vector.tensor_add(mv[:, 1:2], sqp[:, 0:1],
                                             sqp[:, 1:2])
                        msq = stats_pool.tile([128, 1], F32, tag="msq",
                                              name="msq")
                        nc.vector.tensor_mul(msq, mv[:, 0:1], mv[:, 0:1])
                        nc.vector.scalar_tensor_tensor(
                            out=mv[:, 1:2], in0=mv[:, 1:2], scalar=1.0 / N,
                            in1=msq, op0=mybir.AluOpType.mult,
                            op1=mybir.AluOpType.subtract)
                        mean[("x", g, j)] = mv[:, 0:1]
                        var_col[("x", g, j)] = mv[:, 1:2]
                    rstd = stats_pool.tile([128, 1], F32, tag=f"rstd_x{g}{j}",
                                           name=f"rstd_x{g}{j}")
                    nc.scalar.activation(
                        out=rstd, in_=var_col[("x", g, j)],
                        func=mybir.ActivationFunctionType.Sqrt,
                        bias=eps_sb, scale=1.0)
                    nc.vector.reciprocal(out=rstd, in_=rstd)
                    w = ws[("wq", g)]
                    nc.vector.tensor_scalar_mul(w[:, j, :], w[:, j, :], rstd)
                    nc.vector.tensor_scalar_mul(
                        mean_f8[("x", g)][:, j, :], mean[("x", g, j)], MSC)

            # 8. PE: beff_q matmul
            mps_q = sps.tile([128, 1], F32, tag="mps", name="mps")
            for g in range(NG):
                nc.tensor.matmul(mps_q, lhsT=ws[("wq", g)],
                                 rhs=mean_f8[("x", g)], perf_mode=DR,
                                 start=(g == 0), stop=(g == NG - 1))
            nc.sync.dma_start(out=beff_q, in_=bq_d[:, :])
            nc.vector.scalar_tensor_tensor(
                out=beff_q, in0=mps_q, scalar=-1.0 / (WSC * MSC), in1=beff_q,
                op0=mybir.AluOpType.mult, op1=mybir.AluOpType.add)

            # 9. DVE copies for the pre-built vT pairs (run after the
            # stats tail, well before their AV consumers)
            for mp in range(VT_PRE):
                emit_vt_copy(mp, vt_pts.pop(mp))

            stats_scope.close()

            # xres + bo_eff DMAs fire once the gpsimd queue drains the
            # input triggers (~25us) — after the stats DMA window, long
            # before the Wo tail needs them
            nc.gpsimd.dma_start(out=bo_sb, in_=bo_d[:, :])
            nc.gpsimd.dma_start(out=xres, in_=xres_d[:, :])

            if DBG:
                nc.gpsimd.dma_start(out=dbg_k, in_=k_sb)
            conv_scope.close()

            # ---- stage 3: attention + chunked AllGather ----
            # PSUM budget: sT ring 4 + oacc 2 + vt (still open) 2 = 8.
            # The sT ring also serves the q convs and the Wo tail.
            with tc.tile_pool(name="sT", bufs=4, space="PSUM") as sT_pool, \
                 tc.tile_pool(name="oacc", bufs=1, space="PSUM") as oacc_pool, \
                 tc.tile_pool(name="eT", bufs=10) as eT_pool, \
                 tc.tile_pool(name="out_sb", bufs=3) as osb, \
                 tc.tile_pool(name="attn_sm", bufs=4) as sm_pool:

                for nj in range(NT):
                    nsl = slice(512 * nj, 512 * (nj + 1))
                    # q conv for this chunk (ACT applies bias)
                    pq = sT_pool.tile([128, 512], F32, tag="sT", name="sT")
                    for g in range(NG):
                        nc.tensor.matmul(pq, lhsT=ws[("wq", g)],
                                         rhs=xt[g][:, :, nsl], perf_mode=DR,
                                         start=(g == 0), stop=(g == NG - 1))
                    nc.scalar.activation(
                        out=q_sb[:, nsl], in_=pq,
                        func=mybir.ActivationFunctionType.Identity,
                        bias=beff_q, scale=1.0 / WSC)
                    if nj == 0:
                        # 4 more vT pairs fit before the first scores
                        for mp in range(VT_PRE, VT_PRE + 4):
                            vt_pts[mp] = emit_vt_pair(mp)
                            emit_vt_copy(mp, vt_pts.pop(mp))

                    oacc = [oacc_pool.tile([D + 1, 512], F32, tag=f"oacc{h}",
                                           name=f"oacc{h}")
                            for h in range(HPC)]
                    eTs = [[None] * MP for _ in range(HPC)]

                    def av_pair(h, mp):
                        nc.tensor.matmul(
                            oacc[h], lhsT=vT[h][mp][:, :, 0:D + 1],
                            rhs=eTs[h][mp][:, :, :], perf_mode=DR,
                            start=(mp == 0), stop=(mp == MP - 1))

                    for mp in range(MP):
                        if nj == 0 and VT_PRE + 4 + mp < MP:
                            # remaining vT pairs woven into chunk 0; their
                            # AV consumers trail by ~5 pairs
                            vmp = VT_PRE + 4 + mp
                            vt_pts[vmp] = emit_vt_pair(vmp)
                            emit_vt_copy(vmp, vt_pts.pop(vmp))
                        for h in range(HPC):
                            hsl = slice(D * h, D * (h + 1))
                            eT = eT_pool.tile([128, 2, 512], FP8, tag="eT",
                                              name="eT")
                            on_act = (mp * HPC + h) % 24 < ACT_SHARE
                            for j in range(2):
                                m = 2 * mp + j
                                msl = slice(128 * m, 128 * (m + 1))
                                sT = sT_pool.tile([128, 512], F32, tag="sT",
                                                  name="sT")
                                nc.tensor.matmul(
                                    sT, lhsT=k_sb[hsl, msl],
                                    rhs=q_sb[hsl, nsl],
                                    start=True, stop=True)
                                if on_act:
                                    nc.scalar.activation(
                                        out=eT[:, j, :], in_=sT,
                                        func=mybir.ActivationFunctionType.Exp,
                                        scale=SCALE)
                                else:
                                    nc.vector.tensor_scalar(
                                        out=eT.bitcast(U8)[:, j, :], in0=sT,
                                        scalar1=EXPA, scalar2=EXPB,
                                        op0=mybir.AluOpType.mult,
                                        op1=mybir.AluOpType.add)
                            eTs[h][mp] = eT
                            if mp >= 1:
                                av_pair(h, mp - 1)
                    for h in range(HPC):
                        av_pair(h, MP - 1)

                    # o = o_unnorm * (OSC/colsum); the 1/OSC is folded into
                    # the output stt
                    for h in range(HPC):
                        hsl = slice(D * h, D * (h + 1))
                        cs = sm_pool.tile([1, 512], F32, tag="cs", name="cs")
                        nc.scalar.activation(
                            out=cs, in_=oacc[h][D:D + 1, :],
                            func=mybir.ActivationFunctionType.Copy,
                            scale=1.0 / OSC)
                        recip = sm_pool.tile([1, 512], F32, tag="recip",
                                             name="recip")
                        nc.vector.reciprocal_approx_fast(recip, cs)
                        rb = sm_pool.tile([D, 512], F32, tag="rb", name="rb")
                        nc.gpsimd.partition_broadcast(rb, recip)
                        nc.vector.tensor_mul(o_sb[hsl, nsl], oacc[h][0:D, :],
                                             rb)

                    # chunked fp8 AllGather
                    ag_in = dram.tile([CPC, 512], FP8, tag=f"ag_in{nj}",
                                      name=f"ag_in{nj}")
                    ag_out = dram.tile([C, 512], FP8, tag=f"ag_out{nj}",
                                       name=f"ag_out{nj}")
                    nc.sync.dma_start(out=ag_in, in_=o_sb[:, nsl])
                    nc.gpsimd.collective_compute(
                        "AllGather", mybir.AluOpType.bypass,
                        replica_groups=GRPS,
                        ins=[ag_in[:].opt()], outs=[ag_out[:].opt()])
                    # the last chunk's of-DMAs ride the gpsimd queue so the
                    # earlier Wo convs' semaphore thresholds (sync rings)
                    # don't aggregate over them
                    of_q = nc.gpsimd if nj == NT - 1 else nc.sync
                    for g in range(NG):
                        for j in range(2):
                            r0 = 256 * g + 128 * j
                            of_q.dma_start(out=of_sb[nj][g][:, j, :],
                                           in_=ag_out[r0:r0 + 128, :])

                if DBG:
                    nc.gpsimd.dma_start(out=dbg_e, in_=o_sb)
                    nc.gpsimd.dma_start(out=dbg_q, in_=q_sb)

                # xres' = x + bo_eff runs in DVE gaps mid-attention
                nc.vector.tensor_scalar_add(xres, xres, bo_sb)

                # ---- stage 4: ALL Wo convs at the end (collective-
                # dependent; placing them here keeps the attention stream
                # free of AllGather waits)
                for nj in range(NT):
                    nsl = slice(512 * nj, 512 * (nj + 1))
                    po = sT_pool.tile([128, 512], F32, tag="sT", name="sT")
                    for g in range(NG):
                        nc.tensor.matmul(po, lhsT=ws[("wo", g)],
                                         rhs=of_sb[nj][g], perf_mode=DR,
                                         start=(g == 0), stop=(g == NG - 1))
                    ot = osb.tile([128, 512], F32, tag="ot", name="ot")
                    nc.vector.scalar_tensor_tensor(
                        out=ot, in0=po, scalar=1.0 / (WSC * OSC),
                        in1=xres[:, nsl],
                        op0=mybir.AluOpType.mult, op1=mybir.AluOpType.add)
                    nc.scalar.dma_start(out=out_d[:, nsl], in_=ot)
            vt_scope.close()
            acts_scope.close()

    nc.compile()
    return nc


def _shard_inputs(x, s_sty, Wq_w, Wq_b, Wk_w, Wk_b, Wv_w, Wv_b, Wo_w, Wo_b):
    import ml_dtypes
    f8 = ml_dtypes.float8_e4m3
    in_maps = []
    xf = x.reshape(B, C, N)
    sf = s_sty.reshape(B, C, N)
    # fold the v bias through attention + Wo: bo_eff = bo + Wo @ bv
    bo_eff = Wo_b + Wo_w @ Wv_b

    def pack_acts(t):
        # [512, N] -> 2 tiles [128, 2, N], channel = 256g + 128j + p
        r = t.reshape(2, 2, 128, N).transpose(0, 2, 1, 3)
        return [np.ascontiguousarray(r[g].astype(f8)) for g in range(NG)]

    def pack_w(Wt, scale=1.0):
        # W[ch_out_slice, 512].T -> 2 tiles [128, 2, 128]
        r = (Wt.T * scale).reshape(2, 2, 128, CPC).transpose(0, 2, 1, 3)
        return [np.ascontiguousarray(r[g].astype(f8)) for g in range(NG)]

    for core in range(NCORES):
        b, gr = divmod(core, 4)
        ch = slice(CPC * gr, CPC * (gr + 1))
        m = {
            "xres": np.ascontiguousarray(xf[b, ch].astype(
                np.dtype(ml_dtypes.bfloat16))),
            "bq": np.ascontiguousarray(Wq_b[ch, None]),
            "bk": np.ascontiguousarray(Wk_b[ch, None]),
            "bo": np.ascontiguousarray(bo_eff[ch, None].astype(np.float32)),
        }
        for g, t in enumerate(pack_acts(xf[b])):
            m[f"x{g}"] = t
        for g, t in enumerate(pack_acts(sf[b])):
            m[f"s{g}"] = t
        for wname, W in (("wq", Wq_w), ("wk", Wk_w),
                         ("wv", Wv_w), ("wo", Wo_w)):
            for g, t in enumerate(pack_w(W[ch], WSC)):
                m[f"{wname}{g}"] = t
        in_maps.append(m)
    return in_maps


_NC_CACHE = {}


def _get_nc():
    if "nc" not in _NC_CACHE:
        _NC_CACHE["nc"] = _build()
    return _NC_CACHE["nc"]


def run(inputs, trace=False, **kw):
    import time

    from concourse import bass_utils
    nc = _get_nc()
    in_maps = _shard_inputs(**inputs)
    res = None
    for attempt in range(3):
        try:
            res = bass_utils.run_bass_kernel_spmd(
                nc, in_maps, core_ids=list(range(NCORES)), trace=trace, **kw)
            break
        except Exception:
            if attempt == 2:
                raise
            time.sleep(5)
    outs = [np.asarray(res.results[i]["out"]) for i in range(NCORES)]
    full = np.empty((B, C, T, J), np.float32)
    for core in range(NCORES):
        b, gr = divmod(core, 4)
        full[b, CPC * gr:CPC * (gr + 1)] = outs[core].reshape(CPC, T, J)
    return full, res


def kernel(**inputs):
    full, _ = run(inputs, trace=False)
    return full


# revision 26
# speedup vs baseline: 1.3361x; 1.3361x over previous
"""AdaAttention distributed Bass kernel for 8 TRN2 NeuronCores (v4).

Module (per batch b):
  xn = instancenorm(x[b]); sn = instancenorm(s[b])
  q = Wq@xn + bq; k = Wk@sn + bk; v = Wv@s[b] + bv     (1x1 convs, [C, N])
  per head h (d=64): attn = softmax(q_h^T k_h / sqrt(d)) over keys
  o_h = v_h @ attn^T;  out = Wo@o + bo + x[b]

Sharding: core i -> b = i//4, group-rank r = i%4, heads {2r, 2r+1}.

The PE on this part is activity-throttled to ~1.2GHz when kept dense, so
the design minimizes STREAMED COLUMNS and, v4, PE *stall time*:
  - all 1x1 convs run fp8 DoubleRow (K=256 per matmul); x/s arrive from
    the host as fp8 channel-pair packs [128, 2, N], weights as fp8 packs
    [128, 2, 128].
  - vT is produced DIRECTLY as matmul(lhsT=s_pack_tile, rhs=wv_pack):
    out [keys, couts] = v^T tile.  This deletes v_sb, the PE transposes
    and the identity matrix.  bv is folded into bo on the host
    (bo_eff = bo + Wo@bv): the v-bias commutes through the softmax
    column-normalization and the Wo conv.
  - attn@V: fp8 DoubleRow K=256; scores: bf16 K=64.
  - o is scaled x64 before fp8 (dodges fp8e4m3 denormals), AllGathered
    in fp8 per 512-col chunk DURING attention; ALL Wo convs are emitted
    AFTER the last attention chunk so a late peer chunk can never stall
    the PE mid-attention (v3 lost ~106us to exactly that).
  - exp split ScalarE (Exp LUT -> fp8) / VectorE (Schraudolph bit trick
    -> fp8e4m3 bits via uint8 write); colsum row in vT normalizes both.
  - instance norm folded into conv weights (rstd per channel-pair slice,
    beff via a DoubleRow matmul against x256-scaled fp8 means).  Stats
    tail ops are emitted at top priority so k/q convs start ~15-25us.
Host sends big contiguous tensors; DMAs are issued across sync/scalar/
gpsimd queues to cut issue serialization.
"""

import numpy as np

B, C, T, J, H = 2, 512, 128, 24, 8
N = T * J                  # 3072
D = C // H                 # 64
NCORES = 8
GRPS = [[0, 1, 2, 3], [4, 5, 6, 7]]
HPC = 2                    # heads per core
CPC = HPC * D              # 128 channels per core
EPS = 1e-5
SCALE = 1.0 / float(np.sqrt(D))   # 1/8

NT = N // 512              # 6 n-chunks of 512
MT = N // 128              # 24 m-tiles of 128
MP = MT // 2               # 12 m-pairs
NG = 2                     # channel pair-groups (2 x (128x2) = 512)
OSC = 64.0                 # o pre-fp8 scale (1/OSC applied after Wo conv)
WSC = 16.0                 # weight pre-fp8 scale (dodges fp8 denormals)
MSC = 256.0                # mean pre-fp8 scale for the beff matmul

# Schraudolph fast-exp constants for fp8e4m3 output bits (HW-calibrated)
EXPA = float(8.0 * SCALE * np.log2(np.e))
EXPB = 55.593
ACT_SHARE = 14             # of the 24 (head, m-pair) exp units per nj
# Bresenham interleave: spreads the DVE exp units across the chunk
# instead of bunching them at the tail (which stalls the last AVs)
EXP_ON_ACT = [(u * ACT_SHARE) % 24 < ACT_SHARE for u in range(24)]


def _build():
    import os

    import concourse.bass as bass
    import concourse.tile as tile
    from concourse import bacc, mybir

    F32 = mybir.dt.float32
    BF16 = mybir.dt.bfloat16
    FP8 = mybir.dt.float8e4
    U8 = mybir.dt.uint8
    DR = mybir.MatmulPerfMode.DoubleRow
    DBG = os.environ.get("KERNEL_DEBUG") == "1"

    nc = bacc.Bacc("TRN2", target_bir_lowering=False, debug=False,
                   num_devices=NCORES)

    x_d = [nc.dram_tensor(f"x{g}", [128, 2, N], FP8, kind="ExternalInput").ap()
           for g in range(NG)]
    s_d = [nc.dram_tensor(f"s{g}", [128, 2, N], FP8, kind="ExternalInput").ap()
           for g in range(NG)]
    xres_d = nc.dram_tensor("xres", [CPC, N], BF16, kind="ExternalInput").ap()
    w_d = {}
    for wname in ("wq", "wk", "wv", "wo"):
        for g in range(NG):
            w_d[(wname, g)] = nc.dram_tensor(
                f"{wname}{g}", [128, 2, CPC], FP8, kind="ExternalInput").ap()
    bq_d = nc.dram_tensor("bq", [CPC, 1], F32, kind="ExternalInput").ap()
    bk_d = nc.dram_tensor("bk", [CPC, 1], F32, kind="ExternalInput").ap()
    bo_d = nc.dram_tensor("bo", [CPC, 1], F32, kind="ExternalInput").ap()
    out_d = nc.dram_tensor("out", [CPC, N], F32, kind="ExternalOutput").ap()
    if DBG:
        dbg_q = nc.dram_tensor("dbg_q", [CPC, N], F32, kind="ExternalOutput").ap()
        dbg_k = nc.dram_tensor("dbg_k", [CPC, N], F32, kind="ExternalOutput").ap()
        dbg_e = nc.dram_tensor("dbg_e", [CPC, N], F32, kind="ExternalOutput").ap()

    with tile.TileContext(nc) as tc:
        from contextlib import ExitStack
        with tc.tile_pool(name="persist", bufs=1) as persist, \
             tc.tile_pool(name="dram", bufs=1, space="DRAM") as dram:
            acts_scope = ExitStack()
            acts = acts_scope.enter_context(tc.tile_pool(name="acts", bufs=1))
            xt = [acts.tile([128, 2, N], FP8, tag=f"xt{g}", name=f"xt{g}")
                  for g in range(NG)]
            st = [acts.tile([128, 2, N], FP8, tag=f"st{g}", name=f"st{g}")
                  for g in range(NG)]
            scr = acts.tile([128, N], BF16, tag="scr", name="scr")
            q_sb = persist.tile([128, N], BF16, tag="q_sb", name="q_sb")
            k_sb = persist.tile([128, N], BF16, tag="k_sb", name="k_sb")
            vT = [[persist.tile([128, 2, 80], FP8, tag=f"vT{h}_{m}",
                                name=f"vT{h}_{m}") for m in range(MP)]
                  for h in range(HPC)]
            o_sb = persist.tile([128, N], FP8, tag="o_sb", name="o_sb")
            xres = persist.tile([128, N], BF16, tag="xres", name="xres")
            ws = {}
            for wname in ("wq", "wk", "wv", "wo"):
                for g in range(NG):
                    ws[(wname, g)] = persist.tile(
                        [128, 2, CPC], FP8, tag=f"{wname}{g}",
                        name=f"{wname}{g}")
            of_sb = [[persist.tile([128, 2, 512], FP8, tag=f"of{nj}_{g}",
                                   name=f"of{nj}_{g}") for g in range(NG)]
                     for nj in range(NT)]
            beff_q = persist.tile([128, 1], F32, tag="beff_q", name="beff_q")
            beff_k = persist.tile([128, 1], F32, tag="beff_k", name="beff_k")
            bo_sb = persist.tile([128, 1], F32, tag="bo_sb", name="bo_sb")
            eps_sb = persist.tile([128, 1], F32, tag="eps_sb", name="eps_sb")
            warm = persist.tile([128, 1], F32, tag="warm", name="warm")

            nc.vector.memset(o_sb[:, 0:16], 0.0)
            nc.vector.memset(eps_sb, EPS)
            nc.vector.memset(warm, 0.0)
            for h in range(HPC):
                for m in range(MP):
                    nc.vector.memset(vT[h][m][:, :, D:D + 1], 1.0)
            # preload the exp table while the pipe fills
            nc.scalar.activation(out=warm, in_=warm,
                                 func=mybir.ActivationFunctionType.Exp,
                                 scale=1.0)

            # tiny warm-up AllGather: absorbs first-collective ncfw setup
            # and core launch skew; its input DMA leads the sync queue
            wu_in = dram.tile([128, 16], FP8, tag="wu_in", name="wu_in")
            wu_out = dram.tile([512, 16], FP8, tag="wu_out", name="wu_out")
            nc.sync.dma_start(out=wu_in, in_=o_sb[:, 0:16])
            nc.gpsimd.collective_compute(
                "AllGather", mybir.AluOpType.bypass, replica_groups=GRPS,
                ins=[wu_in[:].opt()], outs=[wu_out[:].opt()])

            # input DMAs: s (+wv) first on sync, x on scalar, the rest of
            # the weights after s, xres (needed late) on gpsimd.
            for g in range(NG):
                nc.sync.dma_start(out=ws[("wv", g)], in_=w_d[("wv", g)])
            for c in range(NT):
                csl = slice(512 * c, 512 * (c + 1))
                for g in range(NG):
                    nc.sync.dma_start(out=st[g][:, :, csl],
                                      in_=s_d[g][:, :, csl])
                if c % 3 == 0:
                    # x in 1536-col chunks: matches the ACT stats halves
                    # and gives the DMA engines 3x longer rows
                    xsl = slice(512 * c, 512 * (c + 3))
                    for g in range(NG):
                        nc.scalar.dma_start(out=xt[g][:, :, xsl],
                                            in_=x_d[g][:, :, xsl])
            for wname in ("wk", "wq", "wo"):
                for g in range(NG):
                    nc.sync.dma_start(out=ws[(wname, g)], in_=w_d[(wname, g)])

            # ---- stage 1+2: stats || vT build || k conv.  Emission order
            # == per-engine priority for the Tile scheduler, arranged so
            # the stats tail (aggr/sqrt/fold) preempts bulk work.
            vt_scope = ExitStack()
            vps = vt_scope.enter_context(
                tc.tile_pool(name="vt_ps", bufs=2, space="PSUM"))
            conv_scope = ExitStack()
            cps = conv_scope.enter_context(
                tc.tile_pool(name="conv_ps", bufs=3, space="PSUM"))
            stats_scope = ExitStack()
            stats_pool = stats_scope.enter_context(
                tc.tile_pool(name="stats", bufs=2))
            sps = stats_scope.enter_context(
                tc.tile_pool(name="stats_ps", bufs=2, space="PSUM"))

            mean = {}
            var_col = {}
            rstds = {}

            # 1. DVE: s stats via bn_stats, chunk-trailing the s DMA
            for g in range(NG):
                for j in range(2):
                    stt = stats_pool.tile([128, NT, 6], F32, tag="bn",
                                          name="bn")
                    for c in range(NT):
                        nc.vector.bn_stats(
                            out=stt[:, c, :],
                            in_=st[g][:, j, 512 * c:512 * (c + 1)])
                    mv = stats_pool.tile([128, 2], F32, tag=f"mv_s{g}{j}",
                                         name=f"mv_s{g}{j}")
                    nc.vector.bn_aggr(out=mv, in_=stt)
                    mean[("s", g, j)] = mv[:, 0:1]
                    var_col[("s", g, j)] = mv[:, 1:2]

            # 2. ACT: s rstd sqrts at TOP priority (ready ~when aggr lands)
            for gg in range(NG):
                for jj in range(2):
                    rstd = stats_pool.tile(
                        [128, 1], F32, tag=f"rstd_s{gg}{jj}",
                        name=f"rstd_s{gg}{jj}")
                    nc.scalar.activation(
                        out=rstd, in_=var_col[("s", gg, jj)],
                        func=mybir.ActivationFunctionType.Sqrt,
                        bias=eps_sb, scale=1.0)
                    rstds[("s", gg, jj)] = rstd

            # 3. ACT: x stats for 3 units (2-pass accum, 1536-col chunks)
            ACT_UNITS = [(0, 0), (0, 1), (1, 0)]
            xsump = {}
            xsqp = {}
            for g, j in ACT_UNITS:
                sump = stats_pool.tile([128, 2], F32, tag=f"xsump{g}{j}",
                                       name=f"xsump{g}{j}")
                sqp = stats_pool.tile([128, 2], F32, tag=f"xsqp{g}{j}",
                                      name=f"xsqp{g}{j}")
                for c in range(2):
                    csl = slice(1536 * c, 1536 * (c + 1))
                    nc.scalar.activation(
                        out=scr[:, csl], in_=xt[g][:, j, csl],
                        func=mybir.ActivationFunctionType.Square,
                        accum_out=sqp[:, c:c + 1])
                    nc.scalar.activation(
                        out=scr[:, csl], in_=xt[g][:, j, csl],
                        func=mybir.ActivationFunctionType.Copy,
                        accum_out=sump[:, c:c + 1])
                xsump[(g, j)] = sump
                xsqp[(g, j)] = sqp

            # 4. PE: direct vT build: out[keys, couts] = s_tile^T W_v^T.
            #    lhsT = s pack tile (stationary), rhs = wv pack.  Only the
            #    first VT_PRE pairs are built here (PSUM is scarce and the
            #    DVE copies must not delay the stats tail); pairs VT_PRE..
            #    are interleaved into attention chunk 0, whose AV consumers
            #    trail by ~3 pairs.
            VT_PRE = 2

            def emit_vt_pair(mp):
                pt = vps.tile([128, 2, 128], F32, tag="vt", name="vt")
                for j in range(2):
                    m = 2 * mp + j
                    msl = slice(128 * m, 128 * (m + 1))
                    for g in range(NG):
                        nc.tensor.matmul(pt[:, j, :],
                                         lhsT=st[g][:, :, msl],
                                         rhs=ws[("wv", g)], perf_mode=DR,
                                         start=(g == 0), stop=(g == NG - 1))
                return pt

            def emit_vt_copy(mp, pt):
                for h in range(HPC):
                    nc.vector.tensor_scalar_mul(
                        vT[h][mp][:, :, 0:D], pt[:, :, D * h:D * (h + 1)],
                        1.0 / WSC)

            vt_pts = {mp: emit_vt_pair(mp) for mp in range(VT_PRE)}

            # 5. DVE: s folds + fp8 means; then k conv + biases
            mean_f8 = {}
            for name in ("s", "x"):
                for g in range(NG):
                    mean_f8[(name, g)] = stats_pool.tile(
                        [128, 2, 1], FP8, tag=f"mf_{name}{g}",
                        name=f"mf_{name}{g}")
            for g in range(NG):
                for j in range(2):
                    rstd = rstds[("s", g, j)]
                    nc.vector.reciprocal(out=rstd, in_=rstd)
                    w = ws[("wk", g)]
                    nc.vector.tensor_scalar_mul(w[:, j, :], w[:, j, :], rstd)
                    nc.vector.tensor_scalar_mul(
                        mean_f8[("s", g)][:, j, :], mean[("s", g, j)], MSC)

            # 6. PE: beff_k matmul, then k conv
            mps_k = sps.tile([128, 1], F32, tag="mps", name="mps")
            for g in range(NG):
                nc.tensor.matmul(mps_k, lhsT=ws[("wk", g)],
                                 rhs=mean_f8[("s", g)], perf_mode=DR,
                                 start=(g == 0), stop=(g == NG - 1))
            nc.sync.dma_start(out=beff_k, in_=bk_d[:, :])
            nc.vector.scalar_tensor_tensor(
                out=beff_k, in0=mps_k, scalar=-1.0 / (WSC * MSC), in1=beff_k,
                op0=mybir.AluOpType.mult, op1=mybir.AluOpType.add)
            for nj in range(NT):
                nsl = slice(512 * nj, 512 * (nj + 1))
                pk = cps.tile([128, 512], F32, tag="conv", name="conv")
                for g in range(NG):
                    nc.tensor.matmul(pk, lhsT=ws[("wk", g)],
                                     rhs=st[g][:, :, nsl], perf_mode=DR,
                                     start=(g == 0), stop=(g == NG - 1))
                nc.vector.tensor_scalar(
                    out=k_sb[:, nsl], in0=pk, scalar1=1.0 / WSC,
                    scalar2=beff_k, op0=mybir.AluOpType.mult,
                    op1=mybir.AluOpType.add)

            # 7. x combines (DVE) + x11 stats (DVE) + Sqrt_x (ACT) + folds
            stt = stats_pool.tile([128, NT, 6], F32, tag="bnx", name="bnx")
            for c in range(NT):
                nc.vector.bn_stats(out=stt[:, c, :],
                                   in_=xt[1][:, 1, 512 * c:512 * (c + 1)])
            mv_x11 = stats_pool.tile([128, 2], F32, tag="mv_x11",
                                     name="mv_x11")
            nc.vector.bn_aggr(out=mv_x11, in_=stt)
            mean[("x", 1, 1)] = mv_x11[:, 0:1]
            var_col[("x", 1, 1)] = mv_x11[:, 1:2]

            for g in range(NG):
                for j in range(2):
                    if (g, j) in ACT_UNITS:
                        mv = stats_pool.tile([128, 2], F32,
                                             tag=f"mv_x{g}{j}",
                                             name=f"mv_x{g}{j}")
                        sump, sqp = xsump[(g, j)], xsqp[(g, j)]
                        nc.vector.tensor_add(mv[:, 0:1], sump[:, 0:1],
                                             sump[:, 1:2])
                        nc.vector.tensor_scalar_mul(mv[:, 0:1], mv[:, 0:1],
                                                    1.0 / N)
                        nc.vector.tensor_add(mv[:, 1:2], sqp[:, 0:1],
                                             sqp[:, 1:2])
                        msq = stats_pool.tile([128, 1], F32, tag="msq",
                                              name="msq")
                        nc.vector.tensor_mul(msq, mv[:, 0:1], mv[:, 0:1])
                        nc.vector.scalar_tensor_tensor(
                            out=mv[:, 1:2], in0=mv[:, 1:2], scalar=1.0 / N,
                            in1=msq, op0=mybir.AluOpType.mult,
                            op1=mybir.AluOpType.subtract)
                        mean[("x", g, j)] = mv[:, 0:1]
                        var_col[("x", g, j)] = mv[:, 1:2]
                    rstd = stats_pool.tile([128, 1], F32, tag=f"rstd_x{g}{j}",
                                           name=f"rstd_x{g}{j}")
                    nc.scalar.activation(
                        out=rstd, in_=var_col[("x", g, j)],
                        func=mybir.ActivationFunctionType.Sqrt,
                        bias=eps_sb, scale=1.0)
                    nc.vector.reciprocal(out=rstd, in_=rstd)
                    w = ws[("wq", g)]
                    nc.vector.tensor_scalar_mul(w[:, j, :], w[:, j, :], rstd)
                    nc.vector.tensor_scalar_mul(
                        mean_f8[("x", g)][:, j, :], mean[("x", g, j)], MSC)

            # 8. PE: beff_q matmul
            mps_q = sps.tile([128, 1], F32, tag="mps", name="mps")
            for g in range(NG):
                nc.tensor.matmul(mps_q, lhsT=ws[("wq", g)],
                                 rhs=mean_f8[("x", g)], perf_mode=DR,
                                 start=(g == 0), stop=(g == NG - 1))
            nc.sync.dma_start(out=beff_q, in_=bq_d[:, :])
            nc.vector.scalar_tensor_tensor(
                out=beff_q, in0=mps_q, scalar=-1.0 / (WSC * MSC), in1=beff_q,
                op0=mybir.AluOpType.mult, op1=mybir.AluOpType.add)

            # 9. DVE copies for the pre-built vT pairs (run after the
            # stats tail, well before their AV consumers)
            for mp in range(VT_PRE):
                emit_vt_copy(mp, vt_pts.pop(mp))

            stats_scope.close()

            # xres + bo_eff DMAs fire once the gpsimd queue drains the
            # input triggers (~25us) — after the stats DMA window, long
            # before the Wo tail needs them
            nc.gpsimd.dma_start(out=bo_sb, in_=bo_d[:, :])
            nc.gpsimd.dma_start(out=xres, in_=xres_d[:, :])

            if DBG:
                nc.gpsimd.dma_start(out=dbg_k, in_=k_sb)
            conv_scope.close()

            # ---- stage 3: attention + chunked AllGather ----
            # PSUM budget: sT ring 4 + oacc 2 + vt (still open) 2 = 8.
            # The sT ring also serves the q convs and the Wo tail.
            with tc.tile_pool(name="sT", bufs=4, space="PSUM") as sT_pool, \
                 tc.tile_pool(name="oacc", bufs=1, space="PSUM") as oacc_pool, \
                 tc.tile_pool(name="eT", bufs=10) as eT_pool, \
                 tc.tile_pool(name="out_sb", bufs=3) as osb, \
                 tc.tile_pool(name="attn_sm", bufs=4) as sm_pool:

                for nj in range(NT):
                    nsl = slice(512 * nj, 512 * (nj + 1))
                    # q conv for this chunk (ACT applies bias)
                    pq = sT_pool.tile([128, 512], F32, tag="sT", name="sT")
                    for g in range(NG):
                        nc.tensor.matmul(pq, lhsT=ws[("wq", g)],
                                         rhs=xt[g][:, :, nsl], perf_mode=DR,
                                         start=(g == 0), stop=(g == NG - 1))
                    nc.scalar.activation(
                        out=q_sb[:, nsl], in_=pq,
                        func=mybir.ActivationFunctionType.Identity,
                        bias=beff_q, scale=1.0 / WSC)
                    if nj == 0:
                        # 4 more vT pairs fit before the first scores
                        for mp in range(VT_PRE, VT_PRE + 4):
                            vt_pts[mp] = emit_vt_pair(mp)
                            emit_vt_copy(mp, vt_pts.pop(mp))

                    oacc = [oacc_pool.tile([D + 1, 512], F32, tag=f"oacc{h}",
                                           name=f"oacc{h}")
                            for h in range(HPC)]
                    eTs = [[None] * MP for _ in range(HPC)]

                    def av_pair(h, mp):
                        nc.tensor.matmul(
                            oacc[h], lhsT=vT[h][mp][:, :, 0:D + 1],
                            rhs=eTs[h][mp][:, :, :], perf_mode=DR,
                            start=(mp == 0), stop=(mp == MP - 1))

                    for mp in range(MP):
                        if nj == 0 and VT_PRE + 4 + mp < MP:
                            # remaining vT pairs woven into chunk 0; their
                            # AV consumers trail by ~5 pairs
                            vmp = VT_PRE + 4 + mp
                            vt_pts[vmp] = emit_vt_pair(vmp)
                            emit_vt_copy(vmp, vt_pts.pop(vmp))
                        for h in range(HPC):
                            hsl = slice(D * h, D * (h + 1))
                            eT = eT_pool.tile([128, 2, 512], FP8, tag="eT",
                                              name="eT")
                            on_act = EXP_ON_ACT[mp * HPC + h]
                            for j in range(2):
                                m = 2 * mp + j
                                msl = slice(128 * m, 128 * (m + 1))
                                sT = sT_pool.tile([128, 512], F32, tag="sT",
                                                  name="sT")
                                nc.tensor.matmul(
                                    sT, lhsT=k_sb[hsl, msl],
                                    rhs=q_sb[hsl, nsl],
                                    start=True, stop=True)
                                if on_act:
                                    nc.scalar.activation(
                                        out=eT[:, j, :], in_=sT,
                                        func=mybir.ActivationFunctionType.Exp,
                                        scale=SCALE)
                                else:
                                    nc.vector.tensor_scalar(
                                        out=eT.bitcast(U8)[:, j, :], in0=sT,
                                        scalar1=EXPA, scalar2=EXPB,
                                        op0=mybir.AluOpType.mult,
                                        op1=mybir.AluOpType.add)
                            eTs[h][mp] = eT
                            # AVs trail by 2 pairs: gives the previous
                            # chunk's normalize chain (recip->pb->stt,
                            # ~2.4us) enough runway before this chunk's
                            # first AV needs the oacc bank
                            if mp >= 2:
                                av_pair(h, mp - 2)
                    for d in (MP - 2, MP - 1):
                        for h in range(HPC):
                            av_pair(h, d)

                    # o = o_unnorm * (OSC/colsum); bv folded out on host.
                    # (reciprocal_approx_fast can't read PSUM: ACT stages
                    # the colsum row into SBUF, with 1/OSC folded in)
                    for h in range(HPC):
                        hsl = slice(D * h, D * (h + 1))
                        cs = sm_pool.tile([1, 512], F32, tag="cs", name="cs")
                        nc.scalar.activation(
                            out=cs, in_=oacc[h][D:D + 1, :],
                            func=mybir.ActivationFunctionType.Copy,
                            scale=1.0 / OSC)
                        recip = sm_pool.tile([1, 512], F32, tag="recip",
                                             name="recip")
                        nc.vector.reciprocal_approx_fast(recip, cs)
                        rb = sm_pool.tile([D, 512], F32, tag="rb", name="rb")
                        nc.gpsimd.partition_broadcast(rb, recip)
                        nc.vector.tensor_mul(o_sb[hsl, nsl], oacc[h][0:D, :],
                                             rb)

                    # chunked fp8 AllGather
                    ag_in = dram.tile([CPC, 512], FP8, tag=f"ag_in{nj}",
                                      name=f"ag_in{nj}")
                    ag_out = dram.tile([C, 512], FP8, tag=f"ag_out{nj}",
                                       name=f"ag_out{nj}")
                    nc.sync.dma_start(out=ag_in, in_=o_sb[:, nsl])
                    nc.gpsimd.collective_compute(
                        "AllGather", mybir.AluOpType.bypass,
                        replica_groups=GRPS,
                        ins=[ag_in[:].opt()], outs=[ag_out[:].opt()])
                    # the last chunk's of-DMAs ride the gpsimd queue so the
                    # earlier Wo convs' semaphore thresholds (sync rings)
                    # don't aggregate over them
                    of_q = nc.gpsimd if nj == NT - 1 else nc.sync
                    for g in range(NG):
                        for j in range(2):
                            r0 = 256 * g + 128 * j
                            of_q.dma_start(out=of_sb[nj][g][:, j, :],
                                           in_=ag_out[r0:r0 + 128, :])

                if DBG:
                    nc.gpsimd.dma_start(out=dbg_e, in_=o_sb)
                    nc.gpsimd.dma_start(out=dbg_q, in_=q_sb)

                # xres' = x + bo_eff runs in DVE gaps mid-attention
                nc.vector.tensor_scalar_add(xres, xres, bo_sb)

                # ---- stage 4: ALL Wo convs at the end (collective-
                # dependent; placing them here keeps the attention stream
                # free of AllGather waits)
                for nj in range(NT):
                    nsl = slice(512 * nj, 512 * (nj + 1))
                    po = sT_pool.tile([128, 512], F32, tag="sT", name="sT")
                    for g in range(NG):
                        nc.tensor.matmul(po, lhsT=ws[("wo", g)],
                                         rhs=of_sb[nj][g], perf_mode=DR,
                                         start=(g == 0), stop=(g == NG - 1))
                    ot = osb.tile([128, 512], F32, tag="ot", name="ot")
                    nc.vector.scalar_tensor_tensor(
                        out=ot, in0=po, scalar=1.0 / (WSC * OSC),
                        in1=xres[:, nsl],
                        op0=mybir.AluOpType.mult, op1=mybir.AluOpType.add)
                    nc.scalar.dma_start(out=out_d[:, nsl], in_=ot)
            vt_scope.close()
            acts_scope.close()

    nc.compile()
    return nc


def _shard_inputs(x, s_sty, Wq_w, Wq_b, Wk_w, Wk_b, Wv_w, Wv_b, Wo_w, Wo_b):
    import ml_dtypes
    f8 = ml_dtypes.float8_e4m3
    in_maps = []
    xf = x.reshape(B, C, N)
    sf = s_sty.reshape(B, C, N)
    # fold the v bias through attention + Wo: bo_eff = bo + Wo @ bv
    bo_eff = Wo_b + Wo_w @ Wv_b

    def pack_acts(t):
        # [512, N] -> 2 tiles [128, 2, N], channel = 256g + 128j + p
        r = t.reshape(2, 2, 128, N).transpose(0, 2, 1, 3)
        return [np.ascontiguousarray(r[g].astype(f8)) for g in range(NG)]

    def pack_w(Wt, scale=1.0):
        # W[ch_out_slice, 512].T -> 2 tiles [128, 2, 128]
        r = (Wt.T * scale).reshape(2, 2, 128, CPC).transpose(0, 2, 1, 3)
        return [np.ascontiguousarray(r[g].astype(f8)) for g in range(NG)]

    for core in range(NCORES):
        b, gr = divmod(core, 4)
        ch = slice(CPC * gr, CPC * (gr + 1))
        m = {
            "xres": np.ascontiguousarray(xf[b, ch].astype(
                np.dtype(ml_dtypes.bfloat16))),
            "bq": np.ascontiguousarray(Wq_b[ch, None]),
            "bk": np.ascontiguousarray(Wk_b[ch, None]),
            "bo": np.ascontiguousarray(bo_eff[ch, None].astype(np.float32)),
        }
        for g, t in enumerate(pack_acts(xf[b])):
            m[f"x{g}"] = t
        for g, t in enumerate(pack_acts(sf[b])):
            m[f"s{g}"] = t
        for wname, W in (("wq", Wq_w), ("wk", Wk_w),
                         ("wv", Wv_w), ("wo", Wo_w)):
            for g, t in enumerate(pack_w(W[ch], WSC)):
                m[f"{wname}{g}"] = t
        in_maps.append(m)
    return in_maps


_NC_CACHE = {}


def _get_nc():
    if "nc" not in _NC_CACHE:
        _NC_CACHE["nc"] = _build()
    return _NC_CACHE["nc"]


def run(inputs, trace=False, **kw):
    import time

    from concourse import bass_utils
    nc = _get_nc()
    in_maps = _shard_inputs(**inputs)
    res = None
    for attempt in range(3):
        try:
            res = bass_utils.run_bass_kernel_spmd(
                nc, in_maps, core_ids=list(range(NCORES)), trace=trace, **kw)
            break
        except Exception:
            if attempt == 2:
                raise
            time.sleep(5)
    outs = [np.asarray(res.results[i]["out"]) for i in range(NCORES)]
    full = np.empty((B, C, T, J), np.float32)
    for core in range(NCORES):
        b, gr = divmod(core, 4)
        full[b, CPC * gr:CPC * (gr + 1)] = outs[core].reshape(CPC, T, J)
    return full, res


def kernel(**inputs):
    full, _ = run(inputs, trace=False)
    return full


# revision 51
# speedup vs baseline: 1.3667x; 1.0229x over previous
"""AdaAttention distributed Bass kernel for 8 TRN2 NeuronCores (v4).

Module (per batch b):
  xn = instancenorm(x[b]); sn = instancenorm(s[b])
  q = Wq@xn + bq; k = Wk@sn + bk; v = Wv@s[b] + bv     (1x1 convs, [C, N])
  per head h (d=64): attn = softmax(q_h^T k_h / sqrt(d)) over keys
  o_h = v_h @ attn^T;  out = Wo@o + bo + x[b]

Sharding: core i -> b = i//4, group-rank r = i%4, heads {2r, 2r+1}.

The PE on this part is activity-throttled to ~1.2GHz when kept dense, so
the design minimizes STREAMED COLUMNS and, v4, PE *stall time*:
  - all 1x1 convs run fp8 DoubleRow (K=256 per matmul); x/s arrive from
    the host as fp8 channel-pair packs [128, 2, N], weights as fp8 packs
    [128, 2, 128].
  - vT is produced DIRECTLY as matmul(lhsT=s_pack_tile, rhs=wv_pack):
    out [keys, couts] = v^T tile.  This deletes v_sb, the PE transposes
    and the identity matrix.  bv is folded into bo on the host
    (bo_eff = bo + Wo@bv): the v-bias commutes through the softmax
    column-normalization and the Wo conv.
  - attn@V: fp8 DoubleRow K=256; scores: bf16 K=64.
  - o is scaled x64 before fp8 (dodges fp8e4m3 denormals), AllGathered
    in fp8 per 512-col chunk DURING attention; ALL Wo convs are emitted
    AFTER the last attention chunk so a late peer chunk can never stall
    the PE mid-attention (v3 lost ~106us to exactly that).
  - exp split ScalarE (Exp LUT -> fp8) / VectorE (Schraudolph bit trick
    -> fp8e4m3 bits via uint8 write); colsum row in vT normalizes both.
  - instance norm folded into conv weights (rstd per channel-pair slice,
    beff via a DoubleRow matmul against x256-scaled fp8 means).  Stats
    tail ops are emitted at top priority so k/q convs start ~15-25us.
Host sends big contiguous tensors; DMAs are issued across sync/scalar/
gpsimd queues to cut issue serialization.
"""

import numpy as np

B, C, T, J, H = 2, 512, 128, 24, 8
N = T * J                  # 3072
D = C // H                 # 64
NCORES = 8
GRPS = [[0, 1, 2, 3], [4, 5, 6, 7]]
HPC = 2                    # heads per core
CPC = HPC * D              # 128 channels per core
EPS = 1e-5
SCALE = 1.0 / float(np.sqrt(D))   # 1/8

NT = N // 512              # 6 n-chunks of 512
MT = N // 128              # 24 m-tiles of 128
MP = MT // 2               # 12 m-pairs
NG = 2                     # channel pair-groups (2 x (128x2) = 512)
OSC = 64.0                 # o pre-fp8 scale (1/OSC applied after Wo conv)
WSC = 16.0                 # weight pre-fp8 scale (dodges fp8 denormals)
MSC = 256.0                # mean pre-fp8 scale for the beff matmul

# Schraudolph fast-exp constants for fp8e4m3 output bits (HW-calibrated)
EXPA = float(8.0 * SCALE * np.log2(np.e))
EXPB = 55.593
ACT_SHARE = 14             # of the 24 (head, m-pair) exp units per nj
# Bresenham interleave: spreads the DVE exp units across the chunk
# instead of bunching them at the tail (which stalls the last AVs)
EXP_ON_ACT = [(u * ACT_SHARE) % 24 < ACT_SHARE for u in range(24)]


def _build():
    import os

    import concourse.bass as bass
    import concourse.tile as tile
    from concourse import bacc, mybir

    F32 = mybir.dt.float32
    BF16 = mybir.dt.bfloat16
    FP8 = mybir.dt.float8e4
    U8 = mybir.dt.uint8
    DR = mybir.MatmulPerfMode.DoubleRow
    DBG = os.environ.get("KERNEL_DEBUG") == "1"

    nc = bacc.Bacc("TRN2", target_bir_lowering=False, debug=False,
                   num_devices=NCORES)

    x_d = [nc.dram_tensor(f"x{g}", [128, 2, N], FP8, kind="ExternalInput").ap()
           for g in range(NG)]
    s_d = [nc.dram_tensor(f"s{g}", [128, 2, N], FP8, kind="ExternalInput").ap()
           for g in range(NG)]
    xres_d = nc.dram_tensor("xres", [CPC, N], BF16, kind="ExternalInput").ap()
    w_d = {}
    for wname in ("wq", "wk", "wv", "wo"):
        for g in range(NG):
            w_d[(wname, g)] = nc.dram_tensor(
                f"{wname}{g}", [128, 2, CPC], FP8, kind="ExternalInput").ap()
    bq_d = nc.dram_tensor("bq", [CPC, 1], F32, kind="ExternalInput").ap()
    bk_d = nc.dram_tensor("bk", [CPC, 1], F32, kind="ExternalInput").ap()
    bo_d = nc.dram_tensor("bo", [CPC, 1], F32, kind="ExternalInput").ap()
    out_d = nc.dram_tensor("out", [CPC, N], F32, kind="ExternalOutput").ap()
    if DBG:
        dbg_q = nc.dram_tensor("dbg_q", [CPC, N], F32, kind="ExternalOutput").ap()
        dbg_k = nc.dram_tensor("dbg_k", [CPC, N], F32, kind="ExternalOutput").ap()
        dbg_e = nc.dram_tensor("dbg_e", [CPC, N], F32, kind="ExternalOutput").ap()

    with tile.TileContext(nc) as tc:
        from contextlib import ExitStack
        with tc.tile_pool(name="persist", bufs=1) as persist, \
             tc.tile_pool(name="dram", bufs=1, space="DRAM") as dram:
            acts_scope = ExitStack()
            acts = acts_scope.enter_context(tc.tile_pool(name="acts", bufs=1))
            xt = [acts.tile([128, 2, N], FP8, tag=f"xt{g}", name=f"xt{g}")
                  for g in range(NG)]
            st = [acts.tile([128, 2, N], FP8, tag=f"st{g}", name=f"st{g}")
                  for g in range(NG)]

            def dr_view(t, sl):
                return t[:, :, sl]
            scr = acts.tile([128, N], BF16, tag="scr", name="scr")
            q_sb = persist.tile([128, N], BF16, tag="q_sb", name="q_sb")
            k_sb = persist.tile([128, N], BF16, tag="k_sb", name="k_sb")
            vT = [[persist.tile([128, 2, 80], FP8, tag=f"vT{h}_{m}",
                                name=f"vT{h}_{m}") for m in range(MP)]
                  for h in range(HPC)]
            o_sb = persist.tile([128, N], FP8, tag="o_sb", name="o_sb")
            xres = persist.tile([128, N], BF16, tag="xres", name="xres")
            ws = {}
            for wname in ("wq", "wk", "wv", "wo"):
                for g in range(NG):
                    ws[(wname, g)] = persist.tile(
                        [128, 2, CPC], FP8, tag=f"{wname}{g}",
                        name=f"{wname}{g}")
            of_sb = [[persist.tile([128, 2, 512], FP8, tag=f"of{nj}_{g}",
                                   name=f"of{nj}_{g}") for g in range(NG)]
                     for nj in range(NT)]
            beff_q = persist.tile([128, 1], F32, tag="beff_q", name="beff_q")
            beff_k = persist.tile([128, 1], F32, tag="beff_k", name="beff_k")
            bo_sb = persist.tile([128, 1], F32, tag="bo_sb", name="bo_sb")
            eps_sb = persist.tile([128, 1], F32, tag="eps_sb", name="eps_sb")
            warm = persist.tile([128, 1], F32, tag="warm", name="warm")

            nc.vector.memset(o_sb[:, 0:16], 0.0)
            nc.vector.memset(eps_sb, EPS)
            nc.vector.memset(warm, 0.0)
            for h in range(HPC):
                for m in range(MP):
                    nc.vector.memset(vT[h][m][:, :, D:D + 1], 1.0)
            # preload the exp table while the pipe fills
            nc.scalar.activation(out=warm, in_=warm,
                                 func=mybir.ActivationFunctionType.Exp,
                                 scale=1.0)

            # tiny warm-up AllGather: absorbs first-collective ncfw setup
            # and core launch skew; its input DMA leads the sync queue
            wu_in = dram.tile([128, 16], FP8, tag="wu_in", name="wu_in")
            wu_out = dram.tile([512, 16], FP8, tag="wu_out", name="wu_out")
            nc.sync.dma_start(out=wu_in, in_=o_sb[:, 0:16])
            nc.gpsimd.collective_compute(
                "AllGather", mybir.AluOpType.bypass, replica_groups=GRPS,
                ins=[wu_in[:].opt()], outs=[wu_out[:].opt()])

            # input DMAs: s (+wv) first on sync, x on scalar, the rest of
            # the weights after s, xres (needed late) on gpsimd.
            for g in range(NG):
                nc.sync.dma_start(out=ws[("wv", g)], in_=w_d[("wv", g)])
            # one DMA per (g, j, half): 128 rows of 1536 contiguous bytes
            # each (512-col chunked DMAs had 512B rows and ran the DMA
            # engines at ~30% of roofline), 8 completion events per tensor
            # so the stats still trail
            for w in range(2):
                wsl = slice(1536 * w, 1536 * (w + 1))
                for g in range(NG):
                    for j in range(2):
                        nc.sync.dma_start(out=st[g][:, j, wsl],
                                          in_=s_d[g][:, j, wsl])
                        nc.scalar.dma_start(out=xt[g][:, j, wsl],
                                            in_=x_d[g][:, j, wsl])
            for wname in ("wk", "wq", "wo"):
                for g in range(NG):
                    nc.sync.dma_start(out=ws[(wname, g)], in_=w_d[(wname, g)])

            # ---- stage 1+2: stats || vT build || k conv.  Emission order
            # == per-engine priority for the Tile scheduler, arranged so
            # the stats tail (aggr/sqrt/fold) preempts bulk work.
            vt_scope = ExitStack()
            vps = vt_scope.enter_context(
                tc.tile_pool(name="vt_ps", bufs=2, space="PSUM"))
            conv_scope = ExitStack()
            cps = conv_scope.enter_context(
                tc.tile_pool(name="conv_ps", bufs=3, space="PSUM"))
            stats_scope = ExitStack()
            stats_pool = stats_scope.enter_context(
                tc.tile_pool(name="stats", bufs=2))
            sps = stats_scope.enter_context(
                tc.tile_pool(name="stats_ps", bufs=2, space="PSUM"))

            mean = {}
            var_col = {}
            rstds = {}

            # 1. DVE: s stats via bn_stats (HW caps the free dim at 512),
            # trailing the two DMA waves
            for g in range(NG):
                for j in range(2):
                    stt = stats_pool.tile([128, NT, 6], F32, tag="bn",
                                          name="bn")
                    for c in range(NT):
                        nc.vector.bn_stats(
                            out=stt[:, c, :],
                            in_=st[g][:, j, 512 * c:512 * (c + 1)])
                    mv = stats_pool.tile([128, 2], F32, tag=f"mv_s{g}{j}",
                                         name=f"mv_s{g}{j}")
                    nc.vector.bn_aggr(out=mv, in_=stt)
                    mean[("s", g, j)] = mv[:, 0:1]
                    var_col[("s", g, j)] = mv[:, 1:2]

            # 2. ACT: s rstd sqrts at TOP priority (ready ~when aggr lands)
            for gg in range(NG):
                for jj in range(2):
                    rstd = stats_pool.tile(
                        [128, 1], F32, tag=f"rstd_s{gg}{jj}",
                        name=f"rstd_s{gg}{jj}")
                    nc.scalar.activation(
                        out=rstd, in_=var_col[("s", gg, jj)],
                        func=mybir.ActivationFunctionType.Sqrt,
                        bias=eps_sb, scale=1.0)
                    rstds[("s", gg, jj)] = rstd

            # 3. ACT: x stats for 3 units (2-pass accum, 1536-col waves)
            ACT_UNITS = [(0, 0), (0, 1), (1, 0)]
            xsump = {}
            xsqp = {}
            for g, j in ACT_UNITS:
                sump = stats_pool.tile([128, 2], F32, tag=f"xsump{g}{j}",
                                       name=f"xsump{g}{j}")
                sqp = stats_pool.tile([128, 2], F32, tag=f"xsqp{g}{j}",
                                      name=f"xsqp{g}{j}")
                for c in range(2):
                    csl = slice(1536 * c, 1536 * (c + 1))
                    nc.scalar.activation(
                        out=scr[:, csl], in_=xt[g][:, j, csl],
                        func=mybir.ActivationFunctionType.Square,
                        accum_out=sqp[:, c:c + 1])
                    nc.scalar.activation(
                        out=scr[:, csl], in_=xt[g][:, j, csl],
                        func=mybir.ActivationFunctionType.Copy,
                        accum_out=sump[:, c:c + 1])
                xsump[(g, j)] = sump
                xsqp[(g, j)] = sqp

            # 4. PE: direct vT build: out[keys, couts] = s_tile^T W_v^T.
            #    lhsT = s pack tile (stationary), rhs = wv pack.  Only the
            #    first VT_PRE pairs are built here (PSUM is scarce and the
            #    DVE copies must not delay the stats tail); pairs VT_PRE..
            #    are interleaved into attention chunk 0, whose AV consumers
            #    trail by ~3 pairs.
            VT_PRE = 2

            def emit_vt_pair(mp):
                pt = vps.tile([128, 2, 128], F32, tag="vt", name="vt")
                for j in range(2):
                    m = 2 * mp + j
                    msl = slice(128 * m, 128 * (m + 1))
                    for g in range(NG):
                        nc.tensor.matmul(pt[:, j, :],
                                         lhsT=dr_view(st[g], msl),
                                         rhs=ws[("wv", g)], perf_mode=DR,
                                         start=(g == 0), stop=(g == NG - 1))
                return pt

            def emit_vt_copy(mp, pt):
                for h in range(HPC):
                    nc.vector.tensor_scalar_mul(
                        vT[h][mp][:, :, 0:D], pt[:, :, D * h:D * (h + 1)],
                        1.0 / WSC)

            vt_pts = {mp: emit_vt_pair(mp) for mp in range(VT_PRE)}

            # 5. DVE: s folds + fp8 means; then k conv + biases
            mean_f8 = {}
            for name in ("s", "x"):
                for g in range(NG):
                    mean_f8[(name, g)] = stats_pool.tile(
                        [128, 2, 1], FP8, tag=f"mf_{name}{g}",
                        name=f"mf_{name}{g}")
            for g in range(NG):
                for j in range(2):
                    rstd = rstds[("s", g, j)]
                    nc.vector.reciprocal(out=rstd, in_=rstd)
                    w = ws[("wk", g)]
                    nc.vector.tensor_scalar_mul(w[:, j, :], w[:, j, :], rstd)
                    nc.vector.tensor_scalar_mul(
                        mean_f8[("s", g)][:, j, :], mean[("s", g, j)], MSC)

            # 6. PE: beff_k matmul, then k conv
            mps_k = sps.tile([128, 1], F32, tag="mps", name="mps")
            for g in range(NG):
                nc.tensor.matmul(mps_k, lhsT=ws[("wk", g)],
                                 rhs=mean_f8[("s", g)], perf_mode=DR,
                                 start=(g == 0), stop=(g == NG - 1))
            nc.sync.dma_start(out=beff_k, in_=bk_d[:, :])
            nc.vector.scalar_tensor_tensor(
                out=beff_k, in0=mps_k, scalar=-1.0 / (WSC * MSC), in1=beff_k,
                op0=mybir.AluOpType.mult, op1=mybir.AluOpType.add)
            for nj in range(NT):
                nsl = slice(512 * nj, 512 * (nj + 1))
                pk = cps.tile([128, 512], F32, tag="conv", name="conv")
                for g in range(NG):
                    nc.tensor.matmul(pk, lhsT=ws[("wk", g)],
                                     rhs=dr_view(st[g], nsl), perf_mode=DR,
                                     start=(g == 0), stop=(g == NG - 1))
                nc.vector.tensor_scalar(
                    out=k_sb[:, nsl], in0=pk, scalar1=1.0 / WSC,
                    scalar2=beff_k, op0=mybir.AluOpType.mult,
                    op1=mybir.AluOpType.add)

            # 7. x combines (DVE) + x11 stats (DVE) + Rsqrt_x (ACT) + folds
            stt = stats_pool.tile([128, NT, 6], F32, tag="bnx", name="bnx")
            for c in range(NT):
                nc.vector.bn_stats(out=stt[:, c, :],
                                   in_=xt[1][:, 1, 512 * c:512 * (c + 1)])
            mv_x11 = stats_pool.tile([128, 2], F32, tag="mv_x11",
                                     name="mv_x11")
            nc.vector.bn_aggr(out=mv_x11, in_=stt)
            mean[("x", 1, 1)] = mv_x11[:, 0:1]
            var_col[("x", 1, 1)] = mv_x11[:, 1:2]

            for g in range(NG):
                for j in range(2):
                    if (g, j) in ACT_UNITS:
                        mv = stats_pool.tile([128, 2], F32,
                                             tag=f"mv_x{g}{j}",
                                             name=f"mv_x{g}{j}")
                        sump, sqp = xsump[(g, j)], xsqp[(g, j)]
                        nc.vector.tensor_add(mv[:, 0:1], sump[:, 0:1],
                                             sump[:, 1:2])
                        nc.vector.tensor_scalar_mul(mv[:, 0:1], mv[:, 0:1],
                                                    1.0 / N)
                        nc.vector.tensor_add(mv[:, 1:2], sqp[:, 0:1],
                                             sqp[:, 1:2])
                        msq = stats_pool.tile([128, 1], F32, tag="msq",
                                              name="msq")
                        nc.vector.tensor_mul(msq, mv[:, 0:1], mv[:, 0:1])
                        nc.vector.scalar_tensor_tensor(
                            out=mv[:, 1:2], in0=mv[:, 1:2], scalar=1.0 / N,
                            in1=msq, op0=mybir.AluOpType.mult,
                            op1=mybir.AluOpType.subtract)
                        mean[("x", g, j)] = mv[:, 0:1]
                        var_col[("x", g, j)] = mv[:, 1:2]
                    rstd = stats_pool.tile([128, 1], F32, tag=f"rstd_x{g}{j}",
                                           name=f"rstd_x{g}{j}")
                    nc.scalar.activation(
                        out=rstd, in_=var_col[("x", g, j)],
                        func=mybir.ActivationFunctionType.Sqrt,
                        bias=eps_sb, scale=1.0)
                    nc.vector.reciprocal(out=rstd, in_=rstd)
                    w = ws[("wq", g)]
                    nc.vector.tensor_scalar_mul(w[:, j, :], w[:, j, :], rstd)
                    nc.vector.tensor_scalar_mul(
                        mean_f8[("x", g)][:, j, :], mean[("x", g, j)], MSC)

            # 8. PE: beff_q matmul
            mps_q = sps.tile([128, 1], F32, tag="mps", name="mps")
            for g in range(NG):
                nc.tensor.matmul(mps_q, lhsT=ws[("wq", g)],
                                 rhs=mean_f8[("x", g)], perf_mode=DR,
                                 start=(g == 0), stop=(g == NG - 1))
            nc.sync.dma_start(out=beff_q, in_=bq_d[:, :])
            nc.vector.scalar_tensor_tensor(
                out=beff_q, in0=mps_q, scalar=-1.0 / (WSC * MSC), in1=beff_q,
                op0=mybir.AluOpType.mult, op1=mybir.AluOpType.add)

            # 9. DVE copies for the pre-built vT pairs (run after the
            # stats tail, well before their AV consumers)
            for mp in range(VT_PRE):
                emit_vt_copy(mp, vt_pts.pop(mp))

            stats_scope.close()

            # xres + bo_eff DMAs fire once the gpsimd queue drains the
            # input triggers (~25us) — after the stats DMA window, long
            # before the Wo tail needs them
            nc.gpsimd.dma_start(out=bo_sb, in_=bo_d[:, :])
            nc.gpsimd.dma_start(out=xres, in_=xres_d[:, :])

            if DBG:
                nc.gpsimd.dma_start(out=dbg_k, in_=k_sb)
            conv_scope.close()

            # ---- stage 3: attention + chunked AllGather ----
            # PSUM budget: sT ring 4 + oacc 2 + vt (still open) 2 = 8.
            # The sT ring also serves the q convs and the Wo tail.
            with tc.tile_pool(name="sT", bufs=4, space="PSUM") as sT_pool, \
                 tc.tile_pool(name="oacc", bufs=1, space="PSUM") as oacc_pool, \
                 tc.tile_pool(name="eT", bufs=10) as eT_pool, \
                 tc.tile_pool(name="out_sb", bufs=3) as osb, \
                 tc.tile_pool(name="attn_sm", bufs=4) as sm_pool:

                for nj in range(NT):
                    nsl = slice(512 * nj, 512 * (nj + 1))
                    # q conv for this chunk (ACT applies bias)
                    pq = sT_pool.tile([128, 512], F32, tag="sT", name="sT")
                    for g in range(NG):
                        nc.tensor.matmul(pq, lhsT=ws[("wq", g)],
                                         rhs=dr_view(xt[g], nsl),
                                         perf_mode=DR,
                                         start=(g == 0), stop=(g == NG - 1))
                    nc.scalar.activation(
                        out=q_sb[:, nsl], in_=pq,
                        func=mybir.ActivationFunctionType.Identity,
                        bias=beff_q, scale=1.0 / WSC)
                    if nj == 0:
                        # 4 more vT pairs fit before the first scores
                        for mp in range(VT_PRE, VT_PRE + 4):
                            vt_pts[mp] = emit_vt_pair(mp)
                            emit_vt_copy(mp, vt_pts.pop(mp))

                    oacc = [oacc_pool.tile([D + 1, 512], F32, tag=f"oacc{h}",
                                           name=f"oacc{h}")
                            for h in range(HPC)]
                    eTs = [[None] * MP for _ in range(HPC)]

                    def av_pair(h, mp):
                        nc.tensor.matmul(
                            oacc[h], lhsT=vT[h][mp][:, :, 0:D + 1],
                            rhs=eTs[h][mp][:, :, :], perf_mode=DR,
                            start=(mp == 0), stop=(mp == MP - 1))

                    for mp in range(MP):
                        if nj == 0 and VT_PRE + 4 + mp < MP:
                            # remaining vT pairs woven into chunk 0; their
                            # AV consumers trail by ~5 pairs
                            vmp = VT_PRE + 4 + mp
                            vt_pts[vmp] = emit_vt_pair(vmp)
                            emit_vt_copy(vmp, vt_pts.pop(vmp))
                        for h in range(HPC):
                            hsl = slice(D * h, D * (h + 1))
                            eT = eT_pool.tile([128, 2, 512], FP8, tag="eT",
                                              name="eT")
                            on_act = EXP_ON_ACT[mp * HPC + h]
                            for j in range(2):
                                m = 2 * mp + j
                                msl = slice(128 * m, 128 * (m + 1))
                                sT = sT_pool.tile([128, 512], F32, tag="sT",
                                                  name="sT")
                                nc.tensor.matmul(
                                    sT, lhsT=k_sb[hsl, msl],
                                    rhs=q_sb[hsl, nsl],
                                    start=True, stop=True)
                                if on_act:
                                    nc.scalar.activation(
                                        out=eT[:, j, :], in_=sT,
                                        func=mybir.ActivationFunctionType.Exp,
                                        scale=SCALE)
                                else:
                                    nc.vector.tensor_scalar(
                                        out=eT.bitcast(U8)[:, j, :], in0=sT,
                                        scalar1=EXPA, scalar2=EXPB,
                                        op0=mybir.AluOpType.mult,
                                        op1=mybir.AluOpType.add)
                            eTs[h][mp] = eT
                            # AVs trail by 3 pairs: gives the previous
                            # chunk's normalize chain (cs->recip->pb->mul,
                            # ~3us) enough runway before this chunk's
                            # first AV needs the oacc bank
                            if mp >= 3:
                                av_pair(h, mp - 3)
                    for d in (MP - 3, MP - 2, MP - 1):
                        for h in range(HPC):
                            av_pair(h, d)

                    # o = o_unnorm * (OSC/colsum); bv folded out on host.
                    # The colsum stage-copy rides DVE (keeps the chunk-
                    # boundary ACT burst off the exp-paced sT ring);
                    # reciprocal_approx_fast itself can't read PSUM.
                    for h in range(HPC):
                        hsl = slice(D * h, D * (h + 1))
                        cs = sm_pool.tile([1, 512], F32, tag="cs", name="cs")
                        nc.vector.tensor_scalar_mul(
                            cs, oacc[h][D:D + 1, :], 1.0 / OSC)
                        recip = sm_pool.tile([1, 512], F32, tag="recip",
                                             name="recip")
                        nc.vector.reciprocal_approx_fast(recip, cs)
                        rb = sm_pool.tile([D, 512], F32, tag="rb", name="rb")
                        nc.gpsimd.partition_broadcast(rb, recip)
                        nc.vector.tensor_mul(o_sb[hsl, nsl], oacc[h][0:D, :],
                                             rb)

                    # chunked fp8 AllGather
                    ag_in = dram.tile([CPC, 512], FP8, tag=f"ag_in{nj}",
                                      name=f"ag_in{nj}")
                    ag_out = dram.tile([C, 512], FP8, tag=f"ag_out{nj}",
                                       name=f"ag_out{nj}")
                    nc.sync.dma_start(out=ag_in, in_=o_sb[:, nsl])
                    nc.gpsimd.collective_compute(
                        "AllGather", mybir.AluOpType.bypass,
                        replica_groups=GRPS,
                        ins=[ag_in[:].opt()], outs=[ag_out[:].opt()])
                    # the last chunk's of-DMAs ride the scalar queue so the
                    # earlier Wo convs' semaphore thresholds (sync rings)
                    # don't aggregate over them (gpsimd DMA is ~3x slower)
                    of_q = nc.scalar if nj == NT - 1 else nc.sync
                    for g in range(NG):
                        for j in range(2):
                            r0 = 256 * g + 128 * j
                            of_q.dma_start(out=of_sb[nj][g][:, j, :],
                                           in_=ag_out[r0:r0 + 128, :])

                if DBG:
                    nc.gpsimd.dma_start(out=dbg_e, in_=o_sb)
                    nc.gpsimd.dma_start(out=dbg_q, in_=q_sb)

                # xres' = x + bo_eff runs in DVE gaps mid-attention
                nc.vector.tensor_scalar_add(xres, xres, bo_sb)

                # ---- stage 4: ALL Wo convs at the end (collective-
                # dependent; placing them here keeps the attention stream
                # free of AllGather waits)
                for nj in range(NT):
                    nsl = slice(512 * nj, 512 * (nj + 1))
                    po = sT_pool.tile([128, 512], F32, tag="sT", name="sT")
                    for g in range(NG):
                        nc.tensor.matmul(po, lhsT=ws[("wo", g)],
                                         rhs=of_sb[nj][g], perf_mode=DR,
                                         start=(g == 0), stop=(g == NG - 1))
                    ot = osb.tile([128, 512], F32, tag="ot", name="ot")
                    nc.vector.scalar_tensor_tensor(
                        out=ot, in0=po, scalar=1.0 / (WSC * OSC),
                        in1=xres[:, nsl],
                        op0=mybir.AluOpType.mult, op1=mybir.AluOpType.add)
                    nc.scalar.dma_start(out=out_d[:, nsl], in_=ot)
            vt_scope.close()
            acts_scope.close()

    nc.compile()
    return nc


def _shard_inputs(x, s_sty, Wq_w, Wq_b, Wk_w, Wk_b, Wv_w, Wv_b, Wo_w, Wo_b):
    import ml_dtypes
    f8 = ml_dtypes.float8_e4m3
    in_maps = []
    xf = x.reshape(B, C, N)
    sf = s_sty.reshape(B, C, N)
    # fold the v bias through attention + Wo: bo_eff = bo + Wo @ bv
    bo_eff = Wo_b + Wo_w @ Wv_b

    def pack_acts(t):
        # [512, N] -> 2 tiles [128, 2, N], channel = 256g + 128j + p
        r = t.reshape(2, 2, 128, N).transpose(0, 2, 1, 3)
        return [np.ascontiguousarray(r[g].astype(f8)) for g in range(NG)]

    def pack_w(Wt, scale=1.0):
        # W[ch_out_slice, 512].T -> 2 tiles [128, 2, 128]
        r = (Wt.T * scale).reshape(2, 2, 128, CPC).transpose(0, 2, 1, 3)
        return [np.ascontiguousarray(r[g].astype(f8)) for g in range(NG)]

    for core in range(NCORES):
        b, gr = divmod(core, 4)
        ch = slice(CPC * gr, CPC * (gr + 1))
        m = {
            "xres": np.ascontiguousarray(xf[b, ch].astype(
                np.dtype(ml_dtypes.bfloat16))),
            "bq": np.ascontiguousarray(Wq_b[ch, None]),
            "bk": np.ascontiguousarray(Wk_b[ch, None]),
            "bo": np.ascontiguousarray(bo_eff[ch, None].astype(np.float32)),
        }
        for g, t in enumerate(pack_acts(xf[b])):
            m[f"x{g}"] = t
        for g, t in enumerate(pack_acts(sf[b])):
            m[f"s{g}"] = t
        for wname, W in (("wq", Wq_w), ("wk", Wk_w),
                         ("wv", Wv_w), ("wo", Wo_w)):
            for g, t in enumerate(pack_w(W[ch], WSC)):
                m[f"{wname}{g}"] = t
        in_maps.append(m)
    return in_maps


_NC_CACHE = {}


def _get_nc():
    if "nc" not in _NC_CACHE:
        _NC_CACHE["nc"] = _build()
    return _NC_CACHE["nc"]


def run(inputs, trace=False, **kw):
    import time

    from concourse import bass_utils
    nc = _get_nc()
    in_maps = _shard_inputs(**inputs)
    res = None
    for attempt in range(3):
        try:
            res = bass_utils.run_bass_kernel_spmd(
                nc, in_maps, core_ids=list(range(NCORES)), trace=trace, **kw)
            break
        except Exception:
            if attempt == 2:
                raise
            time.sleep(5)
    outs = [np.asarray(res.results[i]["out"]) for i in range(NCORES)]
    full = np.empty((B, C, T, J), np.float32)
    for core in range(NCORES):
        b, gr = divmod(core, 4)
        full[b, CPC * gr:CPC * (gr + 1)] = outs[core].reshape(CPC, T, J)
    return full, res


def kernel(**inputs):
    full, _ = run(inputs, trace=False)
    return full
